# revision 1
# baseline (speedup 1.0000x reference)
"""Binarized 3x3 conv (BinarizeConv2dSDP) for one TRN2 chip (8 NeuronCores).

Reference computation:
    out = conv2d(sign(x), sign(M), stride=1, pad=1) * Alpha      (all fp32)
    x: (32, 256, 56, 56)   M: (256, 256, 3, 3)   Alpha: (256, 1, 1)

Strategy (data-parallel over batch):
  - Shard x over batch: 4 images per core; replicate M/Alpha on every core.
  - Host ships x as bf16 and M as fp8: IEEE rounding preserves the sign
    bit and the device extracts only the sign bit, so the result is
    bit-identical while input DMA bytes are halved.  M is additionally
    pre-permuted to [C, kk, ot, o2] so the weight DMA lands contraction-
    major and signs land directly in the DoubleRow lhsT layout.
  - Activations are binarized to fp8 into a left-pad-only layout of width
    57: row r+1's leading zero doubles as row r's trailing zero, so every
    3x3 tap is a flat column offset and an 8-row strip is one 455-column
    DoubleRow matmul (contraction 256 channels per pass), 9 taps
    accumulating in PSUM.  Garbage at the 7 row seams is not copied out.
  - The tile framework tracks deps at byte-range granularity, so a
    DoubleRow rhs AP (which spans both channel-half planes) picks up false
    deps on every sign op in between.  Image 0 therefore runs its first
    strips out of small dedicated tiles (A: strip 0, B: strips 1-2) whose
    ranges only cover the signs they truly need; strips 3+ read the main
    tile, whose sign ops all complete early.
  - Engine split: DVE does every sign as a bitwise trick ((msb_byte & 0x80)
    | 0x38 is exactly +/-1.0 in fp8e4; u32-packed for the contiguous weight
    case); ACT does every PSUM evacuation (activation Copy with per-
    partition Alpha as the scale); GpSimd zeroes borders.  No engine ever
    waits on another's queue.
  - DMA entries are issued in need order on the sync HWDGE ring; the DGE
    drains them near-order, and the whole input budget is ~8.6MB in bf16.
  - PE runs dependency-free warmup matmuls so the HAM clock gate is at
    2.4 GHz when the first conv matmul issues; the conv itself is a single
    gapless stream of 504 DoubleRow matmuls at the fp8 throughput floor.
"""

import time

import numpy as np

import concourse.bacc as bacc
import concourse.bass as bass
import concourse.tile as tile
from concourse import mybir
from concourse.bass_utils import run_bass_kernel_spmd

F32 = mybir.dt.float32
BF16 = mybir.dt.bfloat16
FP8 = mybir.dt.float8e4
U8 = mybir.dt.uint8
AND = mybir.AluOpType.bitwise_and
OR = mybir.AluOpType.bitwise_or

# ---- problem geometry (hardcoded; kernel.py must be self-contained) ----
N_CORES = 8
NB = 4            # images per core (32 / 8)
C = 256           # in channels  (2 halves of 128 partitions)
O = 256           # out channels (2 tiles of 128 partitions)
H = W = 56
K = 3
PW = W + 1        # 57: left-pad-only row width
NROWS = H + 2     # storage rows: top pad + 56 + bottom pad
PH = 3312         # NROWS*PW + 1 = 3307 -> multiple of 16 (DoubleRow step)
RS = 8            # output rows per strip
NSTRIP = H // RS  # 7
NCW = (RS - 1) * PW + W   # 455 psum columns per strip matmul
NVAL = RS * W             # 448 valid columns per strip
APH = 864         # tile A: storage rows 0..14 (15*57=855), strip 0
BR0 = 7           # tile B: storage rows 7..27 (21*57=1197), strips 1-2
BPH = 1200
CRT = 14          # rows per image-0 top chunk (a: 0-13, b: 14-27)
CRB = 28          # rows per image-0 bottom chunk (28-55)

WARM = 38         # PE warmup matmuls (N=256 fp8 each)


def build_nc() -> bass.Bass:
    """Build the SPMD Bass program for one core's shard."""
    nc = bacc.Bacc("TRN2")

    x = nc.declare_dram_parameter("x", [NB, C, H, W], BF16, isOutput=False)
    # host-prepermuted weights: m[c, kk, ot, o2] = fp8(M[ot*128+o2, c, kh, kw])
    # (fp8 transport is sign-exact: the device reads only the sign bit)
    m = nc.declare_dram_parameter("m", [C, K * K, 2, 128], FP8, isOutput=False)
    alpha = nc.declare_dram_parameter("alpha", [O], F32, isOutput=False)
    out = nc.declare_dram_parameter("out", [NB, O, H, W], F32, isOutput=True)

    with tile.TileContext(nc) as tc:
        with (
            tc.tile_pool(name="consts", bufs=1) as consts,
            tc.tile_pool(name="wsrc", bufs=2) as wsrc_pool,
            tc.tile_pool(name="xch", bufs=6) as xch_pool,
            tc.tile_pool(name="xin", bufs=3) as xin_pool,
            tc.tile_pool(name="osb", bufs=6) as osb_pool,
            tc.tile_pool(name="pmm", bufs=6, space="PSUM") as pmm_pool,
        ):
            act = consts.tile([128, 2 * NB, PH], FP8)
            acta = consts.tile([128, 2, APH], FP8)
            actb = consts.tile([128, 2, BPH], FP8)
            # wbuf[c2, half, kk*256 + ot*128 + o2]
            #   = sign(M[ot*128+o2, half*128+c2, kh, kw])
            wbuf = consts.tile([128, 2, K * K * O], FP8)
            alpha_sb = consts.tile([128, 2], F32)

            def actv(n, half):  # [rows, cols] view of one padded image half
                return act[:, 2 * n + half, : NROWS * PW].rearrange(
                    "p (r w) -> p r w", w=PW
                )

            # ---- input DMAs (sync HWDGE ring, need order; alpha rides
            # the otherwise-idle GpSimd SWDGE ring so the critical entries
            # get the sync ring's first pipeline slots) ----
            nc.gpsimd.dma_start(
                out=alpha_sb[:], in_=alpha.rearrange("(t o) -> o t", t=2)
            )

            def w_dma(half):
                ws = wsrc_pool.tile([128, K * K * O], FP8)
                nc.sync.dma_start(
                    out=ws[:],
                    in_=m[half * 128 : (half + 1) * 128].rearrange(
                        "c k t o -> c (k t o)"
                    ),
                )
                return (half, ws)

            def x_chunk(half, r0, nr):
                xs = xch_pool.tile([128, CRB * W], BF16)
                nc.sync.dma_start(
                    out=xs[:, : nr * W],
                    in_=x[0, half * 128 : (half + 1) * 128, r0 : r0 + nr, :]
                    .rearrange("c h w -> c (h w)"),
                )
                return xs

            w0 = w_dma(0)
            w1 = w_dma(1)
            # a-chunks carry exactly what tile A needs (input rows 0..9)
            # so the conv-start-critical entries are as small as possible
            xt0a = x_chunk(0, 0, 10)
            xt1a = x_chunk(1, 0, 10)
            xt0b = x_chunk(0, 10, 18)
            xt1b = x_chunk(1, 10, 18)
            xb0 = x_chunk(0, 28, CRB)
            xb1 = x_chunk(1, 28, CRB)
            ximgs = []
            for n in range(1, NB):
                xs = xin_pool.tile([128, 2, H * W], BF16)
                nc.sync.dma_start(
                    out=xs[:], in_=x[n].rearrange("(u c) h w -> c u (h w)", u=2)
                )
                ximgs.append(xs)

            # ---- PE warm-up: dependency-free matmuls so the HAM clock gate
            # reaches 2.4 GHz before the real matmuls start ----
            wz = consts.tile([128, 256], BF16)
            nc.vector.memset(wz[:], 0)
            pwarm = pmm_pool.tile([128, RS * PW], F32, tag="pm")
            for _ in range(WARM):
                nc.tensor.matmul(
                    pwarm[:, :256], wz[:, :128], wz[:], start=True, stop=True
                )

            # ---- borders: zero on GpSimd (otherwise idle) ----
            for n in range(NB):
                a2 = act[:, 2 * n : 2 * n + 2, :]
                nc.gpsimd.memset(a2[:, :, 0:PW], 0)                      # top
                nc.gpsimd.memset(                                        # left
                    a2[:, :, : PW * PW]
                    .rearrange("p a (r w) -> p a r w", w=PW)[:, :, 1:PW, 0:1],
                    0,
                )
                nc.gpsimd.memset(a2[:, :, PW * PW : NROWS * PW + 1], 0)  # bottom
            nc.gpsimd.memset(acta[:, :, 0:PW], 0)
            nc.gpsimd.memset(
                acta[:, :, : 15 * PW]
                .rearrange("p a (r w) -> p a r w", w=PW)[:, :, 1:15, 0:1],
                0,
            )
            nc.gpsimd.memset(
                actb[:, :, : 21 * PW]
                .rearrange("p a (r w) -> p a r w", w=PW)[:, :, 1:21, 0:1],
                0,
            )

            # ---- signs: all on DVE via the fp8 bitwise-sign trick, emitted
            # in need order.  Input row r lands at storage row r+1 (r+1-BR0
            # in tile B). ----
            U32 = mybir.dt.uint32

            def w_sign(half, ws):  # u32-packed: 4 sign bytes per element
                nc.vector.tensor_scalar(
                    wbuf.bitcast(U32)[:, half],
                    ws.bitcast(U32)[:],
                    0x80808080, 0x38383838, op0=AND, op1=OR,
                )

            def sign(dst_rows, xs, r0, nr):  # rows r0..r0+nr-1 of chunk xs
                nc.vector.tensor_scalar(
                    dst_rows.bitcast(U8),
                    xs.bitcast(U8)[:, 2 * r0 * W :]
                    .rearrange("p (h w f) -> p h w f", w=W, f=2)[:, :nr, :, 1:2],
                    0x80, 0x38, op0=AND, op1=OR,
                )

            def tview(t, half, nrows):
                return t[:, half, : nrows * PW].rearrange("p (r w) -> p r w", w=PW)

            w_sign(*w0)
            w_sign(*w1)
            # tile A (strip 0): input rows 0..9 -> A rows 1..10 (strip 0
            # reads storage rows 0..10 only; row 10 is just its left pad)
            sign(tview(acta, 0, 11)[:, 1:11, 1:], xt0a, 0, 10)
            sign(tview(acta, 1, 11)[:, 1:11, 1:], xt1a, 0, 10)
            # tile B (strips 1-2): they read storage rows 8..25 plus row
            # 26's left pad, so input rows 7..24 -> B rows 1..18 suffice
            sign(tview(actb, 0, 21)[:, 1:4, 1:], xt0a, 7, 3)
            sign(tview(actb, 1, 21)[:, 1:4, 1:], xt1a, 7, 3)
            sign(tview(actb, 0, 21)[:, 4:19, 1:], xt0b, 0, 15)
            sign(tview(actb, 1, 21)[:, 4:19, 1:], xt1b, 0, 15)
            # main tile, image 0 (strips 3-6 read storage rows >= 24):
            # input rows 23..27 from the b-chunks, 28..55 from the bottoms
            sign(actv(0, 0)[:, 24:29, 1:], xt0b, 13, 5)
            sign(actv(0, 1)[:, 24:29, 1:], xt1b, 13, 5)
            sign(actv(0, 0)[:, 29 : NROWS - 1, 1:], xb0, 0, CRB)
            sign(actv(0, 1)[:, 29 : NROWS - 1, 1:], xb1, 0, CRB)
            for n in range(1, NB):
                xs = ximgs[n - 1]
                for half in range(2):
                    nc.vector.tensor_scalar(
                        actv(n, half).bitcast(U8)[:, 1 : NROWS - 1, 1:],
                        xs.bitcast(U8)[:, half]
                        .rearrange("p (h w f) -> p h w f", w=W, f=2)[:, :, :, 1:2],
                        0x80, 0x38, op0=AND, op1=OR,
                    )

            # ---- main conv loop: 56 strips x 9 DoubleRow matmuls;
            # ACT evacuates PSUM scaled by per-channel alpha ----
            for n in range(NB):
                for s in range(NSTRIP):
                    for ot in range(2):
                        pm = pmm_pool.tile([128, RS * PW], F32, tag="pm")
                        for kk in range(K * K):
                            kh, kw = divmod(kk, K)
                            if n == 0 and s == 0:
                                rhs = acta[:, :, kh * PW + kw : kh * PW + kw + NCW]
                            elif n == 0 and s < 3:
                                base = (RS * s + kh - BR0) * PW + kw
                                rhs = actb[:, :, base : base + NCW]
                            else:
                                base = (RS * s + kh) * PW + kw
                                rhs = act[:, 2 * n : 2 * n + 2, base : base + NCW]
                            nc.tensor.matmul(
                                pm[:, :NCW],
                                wbuf[:, :, kk * O + ot * 128 : kk * O + ot * 128 + 128],
                                rhs,
                                start=(kk == 0),
                                stop=(kk == K * K - 1),
                                perf_mode=mybir.MatmulPerfMode.DoubleRow,
                            )
                        # evacuate valid columns, scaled by per-channel
                        # alpha; the very last strip goes in two halves so
                        # its out-DMA overlaps its evacuation
                        last = n == NB - 1 and s == NSTRIP - 1 and ot == 1
                        osb = osb_pool.tile([128, NVAL], F32)
                        for r0, r1 in ((0, 4), (4, RS)) if last else ((0, RS),):
                            nc.scalar.mul(
                                osb.rearrange("p (r w) -> p r w", w=W)[:, r0:r1],
                                pm.rearrange("p (r w) -> p r w", w=PW)[:, r0:r1, :W],
                                alpha_sb[:, ot : ot + 1],
                            )
                            nc.sync.dma_start(
                                out=out[
                                    n, ot * 128 : (ot + 1) * 128,
                                    RS * s + r0 : RS * s + r1, :,
                                ].rearrange("o h w -> o (h w)"),
                                in_=osb[:, r0 * W : r1 * W],
                            )
    nc.finalize()
    return nc


_NC_CACHE: dict = {}


def get_nc(*_args) -> bass.Bass:
    if "nc" not in _NC_CACHE:
        _NC_CACHE["nc"] = build_nc()
    return _NC_CACHE["nc"]


def prep_m(M: np.ndarray) -> np.ndarray:
    """Host-side weight permute to [C, kk, ot, o2] in fp8 (layout prep;
    fp8 conversion preserves the sign bit, which is all the device's
    bitwise sign extraction reads, so the result is unchanged)."""
    return np.ascontiguousarray(
        np.asarray(M, dtype=np.float32)
        .reshape(2, 128, C, K, K)
        .transpose(2, 3, 4, 0, 1)
        .reshape(C, K * K, 2, 128)
        .astype(mybir.dt.np(FP8))
    )


def prep_x(x: np.ndarray) -> np.ndarray:
    """Host-side transport compression of x to bf16 (sign-exact)."""
    return np.ascontiguousarray(
        np.asarray(x, dtype=np.float32).astype(mybir.dt.np(BF16))
    )


def kernel(x: np.ndarray, M: np.ndarray, Alpha: np.ndarray) -> np.ndarray:
    """Full (unsharded) inputs in, full output out. Runs on 8 NeuronCores."""
    assert x.shape == (N_CORES * NB, C, H, W), x.shape
    nc = get_nc()
    xb = prep_x(x)
    mt = prep_m(M)
    a = np.ascontiguousarray(np.asarray(Alpha, dtype=np.float32).reshape(O))
    in_maps = [
        {"x": xb[i * NB : (i + 1) * NB], "m": mt, "alpha": a}
        for i in range(N_CORES)
    ]
    last_err = None
    for attempt in range(3):
        try:
            res = run_bass_kernel_spmd(nc, in_maps, list(range(N_CORES)))
            break
        except Exception as e:  # transient NRT/axon faults recover on retry
            last_err = e
            time.sleep(10 * (attempt + 1))
    else:
        raise last_err
    return np.concatenate([res.results[i]["out"] for i in range(N_CORES)], axis=0)

